# revision 21
# baseline (speedup 1.0000x reference)
"""MI-estimator loss kernel for 8 Trainium2 NeuronCores.

Math (reference):
    mu     = relu(x @ w1 + b1) @ w2 + b2
    logvar = tanh(relu(x @ v1 + c1) @ v2 + c2)
    ivar   = exp(-logvar)
    loss   = -0.5/N * sum_{i,d} ivar*(y^2 - 2*mu*y + 2*mu*ybar_d - y2bar_d)

The device computes only the two MLP heads (up to the raw L2 PSUM, no
output biases) and ships raw mu and raw logvar back; the host applies
b2, tanh, exp and all reductions against emb_y in float64. emb_y never
goes to the device, and the serial ACT tanh->exp tail is gone entirely.

Sharding: data-parallel over N=8192 rows -> 1024 rows/core; weights
broadcast. All matmul operands are bf16 (halves DMA bytes; the PE rate
is the same as f32r here); PSUM stays fp32.

Layout: features on partitions. All bf16 inputs live in ONE packed
DRAM tensor loaded as 4 column-range DMAs (the DMA front is HWDGE-
bound at ~625ns/DMA, so fewer+bigger beats many small). L2 outputs are
written with the two n-halves STACKED on PSUM partitions (h0 ->
partitions 0:64, h1 -> 64:128), so each head's result is one (128,512)
tile: one copy op + one DMA out.

Packed tensor pk (128, 3328) bf16, columns:
    0:256      lv_w1[0:128]   (k0)        \  chunk c1 (with x0h0): what
    256:768    xT[0:128, 0:512]   (x0h0)  /  the first matmuls need
    768:1024   mu_w1[0:128]   (k0)        \  chunk c2
    1024:1536  xT[0:128, 512:1024] (x0h1) /
    1536:1792  lv_w1[128:256] (k1)        \
    1792:2048  mu_w1[128:256] (k1)         } chunk c3: the whole k1 pass
    2048:3072  xT[128:256, :]     (x1)    /
    3072:3328  w2 pack (chunk c4): mu_w2[0:128] | lv_w2[0:128] | mu_w2[128:] | lv_w2[128:]
"""

import sys

import numpy as np

try:
    import concourse.bass  # noqa: F401
except ImportError:
    for p in ("/opt/trn_rl_repo", "/root/.axon_site/_ro/trn_rl_repo"):
        if p not in sys.path:
            sys.path.insert(0, p)

N, DX, DY, H = 8192, 256, 64, 256
NCORES = 8
NLOC = N // NCORES  # 1024 rows per core
NH = NLOC // 2  # 512, one PSUM bank of fp32

PK_C = 3328

_CACHE = {}


def _build_nc():
    import concourse.bass as bass
    import concourse.mybir as mybir
    import concourse.tile as tile
    from concourse import bacc
    from concourse.bass import _add_dep_helper

    f32 = mybir.dt.float32
    f16 = mybir.dt.float16
    bf16 = mybir.dt.bfloat16
    AF = mybir.ActivationFunctionType
    ALU = mybir.AluOpType

    nc = bacc.Bacc(
        trn_type="TRN2",
        target_bir_lowering=False,
        debug=False,
        num_devices=NCORES,
    )

    pk = nc.dram_tensor("pk", (128, PK_C), bf16, kind="ExternalInput").ap()
    # bias (128, 4) f32: mu_b1 half0, mu_b1 half1, lv_b1 half0, lv_b1 half1
    bias = nc.dram_tensor("bias", (128, 4), f32, kind="ExternalInput").ap()
    # outputs: stacked (128, 512) f32: partitions 0:64 = n-half0 (rows d),
    # partitions 64:128 = n-half1
    omu = nc.dram_tensor("omu", (128, NH), f16, kind="ExternalOutput").ap()
    olv = nc.dram_tensor("olv", (128, NH), f16, kind="ExternalOutput").ap()

    with tile.TileContext(nc) as tc:
        with (
            tc.tile_pool(name="const", bufs=1) as const,
            tc.tile_pool(name="wk", bufs=1) as wk,
            tc.tile_pool(name="psp", bufs=1, space="PSUM") as psp,
        ):
            # ---- loads: 4 chunks of pk, in PE consumption order ---------
            pk_sb = const.tile([128, PK_C], bf16, tag="pk")
            nc.sync.dma_start(out=pk_sb[:, 0:768], in_=pk[:, 0:768])
            nc.sync.dma_start(out=pk_sb[:, 768:1536], in_=pk[:, 768:1536])
            nc.sync.dma_start(out=pk_sb[:, 1536:3072], in_=pk[:, 1536:3072])
            bias_sb = const.tile([128, 4], f32, tag="bias")
            nc.sync.dma_start(out=bias_sb, in_=bias)
            nc.sync.dma_start(out=pk_sb[:, 3072:PK_C], in_=pk[:, 3072:PK_C])

            W1_OFF = {("lv", 0): 0, ("mu", 0): 768,
                      ("lv", 1): 1536, ("mu", 1): 1792}
            X_OFF = {(0, 0): 256, (0, 1): 1024, (1, 0): 2048, (1, 1): 2560}

            def w1_ap(head, k, m):
                off = W1_OFF[(head, k)] + m * 128
                return pk_sb[:, off : off + 128]

            def x_ap(k, h):
                off = X_OFF[(k, h)]
                return pk_sb[:, off : off + NH]

            def w2_ap(head, k):
                off = 3072 + (2 * k + (0 if head == "mu" else 1)) * DY
                return pk_sb[:, off : off + DY]

            def bias_ap(j, p=128):
                return bias_sb[0:p, j][:, None]

            # One PSUM tensor spanning all 8 banks, sub-ranged manually.
            # Bank map (bank b = cols [512b, 512(b+1))):
            #   b0,b1: L1 lv m0 h0/h1 (b0 then takes L2 lv, stacked 128p)
            #   b2,b3: L1 lv m1; b4,b5: L1 mu m0 (b4 takes L2 mu);
            #   b6,b7: L1 mu m1
            ps_all = psp.tile([128, 8 * NH], f32, tag="ps")

            # Pin PE issue order with no-sync edges (the scheduler otherwise
            # reorders matmuls).
            _prev_mm = [None]

            def mm(out_ap, lhsT, rhs, start, stop):
                m = nc.tensor.matmul(out_ap, lhsT=lhsT, rhs=rhs, start=start,
                                     stop=stop)
                if _prev_mm[0] is not None:
                    _add_dep_helper(m.ins, _prev_mm[0].ins, sync=False,
                                    reason="pin PE order")
                _prev_mm[0] = m
                return m

            # PE warmup: the clock gate holds the PE below 2.4 GHz until it
            # has been busy ~3us; run garbage matmuls while the DMAs load.
            # Results land in bank 0, cleared by the first real accumulation
            # group (start=True).
            _prev_eng = {"act": [None], "dve": [None], "gp": [None]}

            def chain(eng, ins):
                slot = _prev_eng[eng]
                if slot[0] is not None:
                    _add_dep_helper(ins.ins, slot[0].ins, sync=False,
                                    reason=f"pin {eng} order")
                slot[0] = ins

            # warm is never written: the warmup matmuls only need the PE
            # busy, values are irrelevant (bank 0 is cleared by the first
            # real start=True group). Skipping the memset lets warmups begin
            # right after the entry barrier instead of ~330ns later.
            # The PE reaches 2.4 GHz only after ~3000ns of CUMULATIVE
            # matmul busy time; 7 x 512-col + one 32-col warmup at 1.2 GHz
            # = 3016ns, so every real matmul runs at full clock.
            warm = const.tile([128, 306], f32, tag="warm")
            warm_r = warm.bitcast(bf16)
            for _ in range(7):
                mm(ps_all[:, 0:NH], warm_r[:, 0:128], warm_r[:, 0:NH], True,
                   True)
            mm(ps_all[:, 0:32], warm_r[:, 0:128], warm_r[:, 0:32], True, True)

            # ACT table prefetch: the first activation triggers a ~1.3us
            # LoadActFuncSet; fire tiny dummies now (during the DMA wait) so
            # the real relus/copies don't eat it. Relu and Copy both, in
            # case they live in different table sets. They touch only warm
            # cols the warmup matmuls never read (no cross-engine edges).
            for fn in (AF.Relu, AF.Copy):
                d = nc.scalar.activation(out=warm[:, 258:260],
                                         in_=warm[:, 256:258], func=fn)
                chain("act", d)

            l1_base = {("lv", 0): 0, ("lv", 1): 2 * NH,
                       ("mu", 0): 4 * NH, ("mu", 1): 6 * NH}
            GROUPS = [("lv", 0), ("lv", 1), ("mu", 0), ("mu", 1)]

            hT = {}
            for head, m in GROUPS:
                ht = wk.tile([128, NLOC], bf16, tag=f"hT{head}{m}")
                hT[(head, m)] = ht

            # relu engine map: gpsimd cannot read PSUM, so ACT and DVE split
            # the eight halves 4/4.
            RELU_ENG = {("lv", 0, 0): "act", ("lv", 0, 1): "act",
                        ("lv", 1, 0): "dve", ("lv", 1, 1): "dve",
                        ("mu", 0, 0): "act", ("mu", 0, 1): "act",
                        ("mu", 1, 0): "dve", ("mu", 1, 1): "dve"}

            def relu_half(head, m, h):
                base = l1_base[(head, m)]
                ht = hT[(head, m)]
                bias_col = bias_ap((0 if head == "mu" else 2) + m)
                sl = slice(h * NH, (h + 1) * NH)
                ps = ps_all[:, base + h * NH : base + (h + 1) * NH]
                eng = RELU_ENG[(head, m, h)]
                if eng == "act":
                    i = nc.scalar.activation(out=ht[:, sl], in_=ps,
                                             func=AF.Relu, bias=bias_col)
                else:
                    i = nc.vector.tensor_scalar(
                        out=ht[:, sl], in0=ps, scalar1=bias_col, scalar2=0.0,
                        op0=ALU.add, op1=ALU.max)
                chain(eng, i)

            def l1_mm(head, m, k, h):
                base = l1_base[(head, m)]
                mm(ps_all[:, base + h * NH : base + (h + 1) * NH],
                   w1_ap(head, k, m), x_ap(k, h), k == 0, k == 1)

            # L2 outputs, n-halves stacked on partitions: h0 -> rows 0:64,
            # h1 -> rows 64:128. lv -> bank 0, mu -> bank 4.
            L2_BANK = {"lv": 0, "mu": 4 * NH}

            def l2_half(head, h):
                base = L2_BANK[head]
                out = ps_all[h * DY : (h + 1) * DY, base : base + NH]
                for k in range(2):
                    mm(out, w2_ap(head, k),
                       hT[(head, k)][:, h * NH : (h + 1) * NH],
                       k == 0, k == 1)

            # ---- k0 pass: stream behind the c1/c2 chunk DMAs ------------
            for head, m in [("lv", 0), ("lv", 1), ("mu", 0), ("mu", 1)]:
                l1_mm(head, m, 0, 0)
            for head, m in [("lv", 0), ("lv", 1), ("mu", 0), ("mu", 1)]:
                l1_mm(head, m, 0, 1)
            # ---- k1 pass: lv groups first so L2 lv (and its copy) is early.
            # The DVE-fed groups (m=1) go first within each head: DVE is the
            # slower relu engine, so its chain must start soonest.
            l1_mm("lv", 1, 1, 0)
            l1_mm("lv", 0, 1, 0)
            l1_mm("lv", 1, 1, 1)
            l1_mm("lv", 0, 1, 1)
            relu_half("lv", 1, 0)
            relu_half("lv", 0, 0)
            relu_half("lv", 1, 1)
            relu_half("lv", 0, 1)
            l1_mm("mu", 1, 1, 0)
            l1_mm("mu", 0, 1, 0)
            l1_mm("mu", 1, 1, 1)
            l1_mm("mu", 0, 1, 1)
            relu_half("mu", 1, 0)
            relu_half("mu", 0, 0)
            relu_half("mu", 1, 1)
            relu_half("mu", 0, 1)
            # ---- L2 -----------------------------------------------------
            l2_half("lv", 0)
            l2_half("lv", 1)
            l2_half("mu", 0)
            l2_half("mu", 1)
            lv_ps = ps_all[:, L2_BANK["lv"] : L2_BANK["lv"] + NH]
            mu_ps = ps_all[:, L2_BANK["mu"] : L2_BANK["mu"] + NH]

            # ---- tail: PSUM -> SBUF fp16 copies + DMA out ---------------
            # fp16 keeps ~3 more mantissa bits than bf16 (raw mu/logvar are
            # O(1), far inside fp16 range) and halves the out transfers.
            # lv copy on DVE (its relus end first), mu copy on ACT.
            lv_sb = wk.tile([128, NH], f16, tag="lv_sb")
            mu_sb = wk.tile([128, NH], f16, tag="mu_sb")
            i = nc.vector.tensor_scalar(
                out=lv_sb, in0=lv_ps, scalar1=0.0, scalar2=0.0,
                op0=ALU.add, op1=ALU.bypass)
            chain("dve", i)
            nc.sync.dma_start(out=olv, in_=lv_sb)
            i = nc.scalar.activation(out=mu_sb, in_=mu_ps, func=AF.Copy)
            chain("act", i)
            # issue omu's DMA from the ACT queue right behind the copy: no
            # SEQ re-dispatch latency before its HWDGE stage
            nc.scalar.dma_start(out=omu, in_=mu_sb)

    nc.compile()
    return nc


def _get_nc():
    if "nc" not in _CACHE:
        _CACHE["nc"] = _build_nc()
    return _CACHE["nc"]


def _make_in_maps(inputs):
    import ml_dtypes

    bf16 = ml_dtypes.bfloat16
    # convert everything to numpy up front: slicing jax arrays here could
    # otherwise dispatch to the (axon) device backend
    emb_x = np.asarray(inputs["emb_x"], dtype=np.float32)

    mu_w1 = np.asarray(inputs["mu_w1"], np.float32)
    lv_w1 = np.asarray(inputs["lv_w1"], np.float32)
    mu_w2 = np.asarray(inputs["mu_w2"], np.float32)
    lv_w2 = np.asarray(inputs["lv_w2"], np.float32)

    bias = np.zeros((128, 4), dtype=np.float32)
    bias[:, 0] = np.asarray(inputs["mu_b1"][:128], np.float32)
    bias[:, 1] = np.asarray(inputs["mu_b1"][128:], np.float32)
    bias[:, 2] = np.asarray(inputs["lv_b1"][:128], np.float32)
    bias[:, 3] = np.asarray(inputs["lv_b1"][128:], np.float32)

    w2pack = np.concatenate(
        [mu_w2[0:128], lv_w2[0:128], mu_w2[128:256], lv_w2[128:256]], axis=1
    )  # (128, 256)

    in_maps = []
    for c in range(NCORES):
        rows = slice(c * NLOC, (c + 1) * NLOC)
        xT = emb_x[rows].T  # (256, 1024)
        pk = np.concatenate(
            [
                lv_w1[0:128],
                xT[0:128, 0:NH],
                mu_w1[0:128],
                xT[0:128, NH:NLOC],
                lv_w1[128:256],
                mu_w1[128:256],
                xT[128:256, :],
                w2pack,
            ],
            axis=1,
        )  # (128, 3328)
        in_maps.append(
            {
                "pk": np.ascontiguousarray(pk.astype(bf16)),
                "bias": bias,
            }
        )
    return in_maps


def kernel(emb_x, emb_y, mu_w1, mu_b1, mu_w2, mu_b2, lv_w1, lv_b1, lv_w2, lv_b2):
    from concourse.bass_utils import run_bass_kernel_spmd

    emb_y = np.asarray(emb_y, dtype=np.float32)
    in_maps = _make_in_maps(
        {
            "emb_x": emb_x,
            "mu_w1": mu_w1,
            "mu_b1": mu_b1,
            "mu_w2": mu_w2,
            "lv_w1": lv_w1,
            "lv_b1": lv_b1,
            "lv_w2": lv_w2,
        }
    )

    nc = _get_nc()
    res = run_bass_kernel_spmd(nc, in_maps, list(range(NCORES)))

    b2mu = np.asarray(mu_b2, np.float64)  # (64,)
    b2lv = np.asarray(lv_b2, np.float64)
    B = np.zeros(DY)
    E = np.zeros(DY)
    A = 0.0
    C = 0.0
    for c in range(NCORES):
        yT = emb_y[c * NLOC : (c + 1) * NLOC].T.astype(np.float64)  # (64,1024)
        mu_st = res.results[c]["omu"].astype(np.float64)  # (128, 512)
        lv_st = res.results[c]["olv"].astype(np.float64)
        # unstack: partitions 0:64 = n cols 0:512, 64:128 = cols 512:1024
        mu = np.concatenate([mu_st[0:DY], mu_st[DY:]], axis=1) + b2mu[:, None]
        lv_raw = np.concatenate([lv_st[0:DY], lv_st[DY:]], axis=1)
        ivc = np.exp(-np.tanh(lv_raw + b2lv[:, None]))
        mic = mu * ivc
        B += ivc.sum(axis=1)
        E += mic.sum(axis=1)
        A += (ivc * yT**2).sum()
        C += (mic * yT).sum()

    y64 = emb_y.astype(np.float64)
    ybar = y64.mean(axis=0)
    y2bar = (y64**2).mean(axis=0)

    total = A - 2.0 * C + (2.0 * E * ybar - B * y2bar).sum()
    loss = -0.5 / N * total
    return np.float32(loss)


# revision 22
# speedup vs baseline: 1.0227x; 1.0227x over previous
"""MI-estimator loss kernel for 8 Trainium2 NeuronCores.

Math (reference):
    mu     = relu(x @ w1 + b1) @ w2 + b2
    logvar = tanh(relu(x @ v1 + c1) @ v2 + c2)
    ivar   = exp(-logvar)
    loss   = -0.5/N * sum_{i,d} ivar*(y^2 - 2*mu*y + 2*mu*ybar_d - y2bar_d)

The device computes only the two MLP heads (up to the raw L2 PSUM, no
output biases) and ships raw mu and raw logvar back; the host applies
b2, tanh, exp and all reductions against emb_y in float64. emb_y never
goes to the device, and the serial ACT tanh->exp tail is gone entirely.

Sharding: data-parallel over N=8192 rows -> 1024 rows/core; weights
broadcast. All matmul operands are bf16 (halves DMA bytes; the PE rate
is the same as f32r here); PSUM stays fp32.

Layout: features on partitions. All bf16 inputs live in ONE packed
DRAM tensor loaded as 4 column-range DMAs (the DMA front is HWDGE-
bound at ~625ns/DMA, so fewer+bigger beats many small). L2 outputs are
written with the two n-halves STACKED on PSUM partitions (h0 ->
partitions 0:64, h1 -> 64:128), so each head's result is one (128,512)
tile: one copy op + one DMA out.

Packed tensor pk (128, 3328) bf16, columns:
    0:256      lv_w1[0:128]   (k0)        \  chunk c1 (with x0h0): what
    256:768    xT[0:128, 0:512]   (x0h0)  /  the first matmuls need
    768:1024   mu_w1[0:128]   (k0)        \  chunk c2
    1024:1536  xT[0:128, 512:1024] (x0h1) /
    1536:1792  lv_w1[128:256] (k1)        \
    1792:2048  mu_w1[128:256] (k1)         } chunk c3: the whole k1 pass
    2048:3072  xT[128:256, :]     (x1)    /
    3072:3328  w2 pack (chunk c4): mu_w2[0:128] | lv_w2[0:128] | mu_w2[128:] | lv_w2[128:]
"""

import sys

import numpy as np

try:
    import concourse.bass  # noqa: F401
except ImportError:
    for p in ("/opt/trn_rl_repo", "/root/.axon_site/_ro/trn_rl_repo"):
        if p not in sys.path:
            sys.path.insert(0, p)

N, DX, DY, H = 8192, 256, 64, 256
NCORES = 8
NLOC = N // NCORES  # 1024 rows per core
NH = NLOC // 2  # 512, one PSUM bank of fp32

PK_C = 3328

_CACHE = {}


def _build_nc():
    import concourse.bass as bass
    import concourse.mybir as mybir
    import concourse.tile as tile
    from concourse import bacc
    from concourse.bass import _add_dep_helper

    f32 = mybir.dt.float32
    f16 = mybir.dt.float16
    bf16 = mybir.dt.bfloat16
    AF = mybir.ActivationFunctionType
    ALU = mybir.AluOpType

    nc = bacc.Bacc(
        trn_type="TRN2",
        target_bir_lowering=False,
        debug=False,
        num_devices=NCORES,
    )

    pk = nc.dram_tensor("pk", (128, PK_C), bf16, kind="ExternalInput").ap()
    # bias (128, 4) f32: mu_b1 half0, mu_b1 half1, lv_b1 half0, lv_b1 half1
    bias = nc.dram_tensor("bias", (128, 4), f32, kind="ExternalInput").ap()
    # outputs: stacked (128, 512) f32: partitions 0:64 = n-half0 (rows d),
    # partitions 64:128 = n-half1
    omu = nc.dram_tensor("omu", (128, NH), f16, kind="ExternalOutput").ap()
    olv = nc.dram_tensor("olv", (128, NH), f16, kind="ExternalOutput").ap()

    with tile.TileContext(nc) as tc:
        with (
            tc.tile_pool(name="const", bufs=1) as const,
            tc.tile_pool(name="wk", bufs=1) as wk,
            tc.tile_pool(name="psp", bufs=1, space="PSUM") as psp,
        ):
            # ---- loads: 4 chunks of pk, in PE consumption order ---------
            pk_sb = const.tile([128, PK_C], bf16, tag="pk")
            nc.sync.dma_start(out=pk_sb[:, 0:768], in_=pk[:, 0:768])
            nc.sync.dma_start(out=pk_sb[:, 768:1536], in_=pk[:, 768:1536])
            nc.sync.dma_start(out=pk_sb[:, 1536:3072], in_=pk[:, 1536:3072])
            bias_sb = const.tile([128, 4], f32, tag="bias")
            nc.sync.dma_start(out=bias_sb, in_=bias)
            nc.sync.dma_start(out=pk_sb[:, 3072:PK_C], in_=pk[:, 3072:PK_C])

            W1_OFF = {("lv", 0): 0, ("mu", 0): 768,
                      ("lv", 1): 1536, ("mu", 1): 1792}
            X_OFF = {(0, 0): 256, (0, 1): 1024, (1, 0): 2048, (1, 1): 2560}

            def w1_ap(head, k, m):
                off = W1_OFF[(head, k)] + m * 128
                return pk_sb[:, off : off + 128]

            def x_ap(k, h):
                off = X_OFF[(k, h)]
                return pk_sb[:, off : off + NH]

            def w2_ap(head, k):
                off = 3072 + (2 * k + (0 if head == "mu" else 1)) * DY
                return pk_sb[:, off : off + DY]

            def bias_ap(j, p=128):
                return bias_sb[0:p, j][:, None]

            # One PSUM tensor spanning all 8 banks, sub-ranged manually.
            # Bank map (bank b = cols [512b, 512(b+1))):
            #   b0,b1: L1 lv m0 h0/h1 (b0 then takes L2 lv, stacked 128p)
            #   b2,b3: L1 lv m1; b4,b5: L1 mu m0 (b4 takes L2 mu);
            #   b6,b7: L1 mu m1
            ps_all = psp.tile([128, 8 * NH], f32, tag="ps")

            # Pin PE issue order with no-sync edges (the scheduler otherwise
            # reorders matmuls).
            _prev_mm = [None]

            def mm(out_ap, lhsT, rhs, start, stop):
                m = nc.tensor.matmul(out_ap, lhsT=lhsT, rhs=rhs, start=start,
                                     stop=stop)
                if _prev_mm[0] is not None:
                    _add_dep_helper(m.ins, _prev_mm[0].ins, sync=False,
                                    reason="pin PE order")
                _prev_mm[0] = m
                return m

            # PE warmup: the clock gate holds the PE below 2.4 GHz until it
            # has been busy ~3us; run garbage matmuls while the DMAs load.
            # Results land in bank 0, cleared by the first real accumulation
            # group (start=True).
            _prev_eng = {"act": [None], "dve": [None], "gp": [None]}

            def chain(eng, ins):
                slot = _prev_eng[eng]
                if slot[0] is not None:
                    _add_dep_helper(ins.ins, slot[0].ins, sync=False,
                                    reason=f"pin {eng} order")
                slot[0] = ins

            # warm is never written: the warmup matmuls only need the PE
            # busy, values are irrelevant (bank 0 is cleared by the first
            # real start=True group). Skipping the memset lets warmups begin
            # right after the entry barrier instead of ~330ns later.
            # The PE p-state is evaluated at DISPATCH time: matmuls
            # dispatched before busy_start+3us run at 1.2 GHz no matter how
            # much warmup ran. Warmups pin busy_start early and keep the PE
            # fed until the first chunk lands; the first two real matmuls
            # (dispatched at the c1 semaphore, ~2.7us after busy_start)
            # unavoidably run mid-speed.
            warm = const.tile([128, 306], f32, tag="warm")
            warm_r = warm.bitcast(bf16)
            for _ in range(6):
                mm(ps_all[:, 0:NH], warm_r[:, 0:128], warm_r[:, 0:NH], True,
                   True)

            # ACT table prefetch: the first activation triggers a ~1.3us
            # LoadActFuncSet; fire tiny dummies now (during the DMA wait) so
            # the real relus/copies don't eat it. Relu and Copy both, in
            # case they live in different table sets. They touch only warm
            # cols the warmup matmuls never read (no cross-engine edges).
            for fn in (AF.Relu, AF.Copy):
                d = nc.scalar.activation(out=warm[:, 258:260],
                                         in_=warm[:, 256:258], func=fn)
                chain("act", d)

            l1_base = {("lv", 0): 0, ("lv", 1): 2 * NH,
                       ("mu", 0): 4 * NH, ("mu", 1): 6 * NH}
            GROUPS = [("lv", 0), ("lv", 1), ("mu", 0), ("mu", 1)]

            hT = {}
            for head, m in GROUPS:
                ht = wk.tile([128, NLOC], bf16, tag=f"hT{head}{m}")
                hT[(head, m)] = ht

            # relu engine map: gpsimd cannot read PSUM, so ACT and DVE split
            # the eight halves 4/4.
            RELU_ENG = {("lv", 0, 0): "act", ("lv", 0, 1): "act",
                        ("lv", 1, 0): "dve", ("lv", 1, 1): "dve",
                        ("mu", 0, 0): "act", ("mu", 0, 1): "act",
                        ("mu", 1, 0): "dve", ("mu", 1, 1): "dve"}

            def relu_half(head, m, h):
                base = l1_base[(head, m)]
                ht = hT[(head, m)]
                bias_col = bias_ap((0 if head == "mu" else 2) + m)
                sl = slice(h * NH, (h + 1) * NH)
                ps = ps_all[:, base + h * NH : base + (h + 1) * NH]
                eng = RELU_ENG[(head, m, h)]
                if eng == "act":
                    i = nc.scalar.activation(out=ht[:, sl], in_=ps,
                                             func=AF.Relu, bias=bias_col)
                else:
                    i = nc.vector.tensor_scalar(
                        out=ht[:, sl], in0=ps, scalar1=bias_col, scalar2=0.0,
                        op0=ALU.add, op1=ALU.max)
                chain(eng, i)

            def l1_mm(head, m, k, h):
                base = l1_base[(head, m)]
                mm(ps_all[:, base + h * NH : base + (h + 1) * NH],
                   w1_ap(head, k, m), x_ap(k, h), k == 0, k == 1)

            # L2 outputs, n-halves stacked on partitions: h0 -> rows 0:64,
            # h1 -> rows 64:128. lv -> bank 0, mu -> bank 4.
            L2_BANK = {"lv": 0, "mu": 4 * NH}

            def l2_half(head, h):
                base = L2_BANK[head]
                out = ps_all[h * DY : (h + 1) * DY, base : base + NH]
                for k in range(2):
                    mm(out, w2_ap(head, k),
                       hT[(head, k)][:, h * NH : (h + 1) * NH],
                       k == 0, k == 1)

            # ---- k0 pass: stream behind the c1/c2 chunk DMAs ------------
            for head, m in [("lv", 0), ("lv", 1), ("mu", 0), ("mu", 1)]:
                l1_mm(head, m, 0, 0)
            for head, m in [("lv", 0), ("lv", 1), ("mu", 0), ("mu", 1)]:
                l1_mm(head, m, 0, 1)
            # ---- k1 pass: lv groups first so L2 lv (and its copy) is early.
            # The DVE-fed groups (m=1) go first within each head: DVE is the
            # slower relu engine, so its chain must start soonest.
            l1_mm("lv", 1, 1, 0)
            l1_mm("lv", 0, 1, 0)
            l1_mm("lv", 1, 1, 1)
            l1_mm("lv", 0, 1, 1)
            relu_half("lv", 1, 0)
            relu_half("lv", 0, 0)
            relu_half("lv", 1, 1)
            relu_half("lv", 0, 1)
            l1_mm("mu", 1, 1, 0)
            l1_mm("mu", 0, 1, 0)
            l1_mm("mu", 1, 1, 1)
            l1_mm("mu", 0, 1, 1)
            relu_half("mu", 1, 0)
            relu_half("mu", 0, 0)
            relu_half("mu", 1, 1)
            relu_half("mu", 0, 1)
            # ---- L2 -----------------------------------------------------
            l2_half("lv", 0)
            l2_half("lv", 1)
            l2_half("mu", 0)
            l2_half("mu", 1)
            lv_ps = ps_all[:, L2_BANK["lv"] : L2_BANK["lv"] + NH]
            mu_ps = ps_all[:, L2_BANK["mu"] : L2_BANK["mu"] + NH]

            # ---- tail: PSUM -> SBUF fp16 copies + DMA out ---------------
            # fp16 keeps ~3 more mantissa bits than bf16 (raw mu/logvar are
            # O(1), far inside fp16 range) and halves the out transfers.
            # lv copy on DVE (its relus end first), mu copy on ACT.
            lv_sb = wk.tile([128, NH], f16, tag="lv_sb")
            mu_sb = wk.tile([128, NH], f16, tag="mu_sb")
            i = nc.vector.tensor_scalar(
                out=lv_sb, in0=lv_ps, scalar1=0.0, scalar2=0.0,
                op0=ALU.add, op1=ALU.bypass)
            chain("dve", i)
            nc.sync.dma_start(out=olv, in_=lv_sb)
            i = nc.scalar.activation(out=mu_sb, in_=mu_ps, func=AF.Copy)
            chain("act", i)
            # issue omu's DMA from the ACT queue right behind the copy: no
            # SEQ re-dispatch latency before its HWDGE stage
            nc.scalar.dma_start(out=omu, in_=mu_sb)

    nc.compile()
    return nc


def _get_nc():
    if "nc" not in _CACHE:
        _CACHE["nc"] = _build_nc()
    return _CACHE["nc"]


def _make_in_maps(inputs):
    import ml_dtypes

    bf16 = ml_dtypes.bfloat16
    # convert everything to numpy up front: slicing jax arrays here could
    # otherwise dispatch to the (axon) device backend
    emb_x = np.asarray(inputs["emb_x"], dtype=np.float32)

    mu_w1 = np.asarray(inputs["mu_w1"], np.float32)
    lv_w1 = np.asarray(inputs["lv_w1"], np.float32)
    mu_w2 = np.asarray(inputs["mu_w2"], np.float32)
    lv_w2 = np.asarray(inputs["lv_w2"], np.float32)

    bias = np.zeros((128, 4), dtype=np.float32)
    bias[:, 0] = np.asarray(inputs["mu_b1"][:128], np.float32)
    bias[:, 1] = np.asarray(inputs["mu_b1"][128:], np.float32)
    bias[:, 2] = np.asarray(inputs["lv_b1"][:128], np.float32)
    bias[:, 3] = np.asarray(inputs["lv_b1"][128:], np.float32)

    w2pack = np.concatenate(
        [mu_w2[0:128], lv_w2[0:128], mu_w2[128:256], lv_w2[128:256]], axis=1
    )  # (128, 256)

    in_maps = []
    for c in range(NCORES):
        rows = slice(c * NLOC, (c + 1) * NLOC)
        xT = emb_x[rows].T  # (256, 1024)
        pk = np.concatenate(
            [
                lv_w1[0:128],
                xT[0:128, 0:NH],
                mu_w1[0:128],
                xT[0:128, NH:NLOC],
                lv_w1[128:256],
                mu_w1[128:256],
                xT[128:256, :],
                w2pack,
            ],
            axis=1,
        )  # (128, 3328)
        in_maps.append(
            {
                "pk": np.ascontiguousarray(pk.astype(bf16)),
                "bias": bias,
            }
        )
    return in_maps


def kernel(emb_x, emb_y, mu_w1, mu_b1, mu_w2, mu_b2, lv_w1, lv_b1, lv_w2, lv_b2):
    from concourse.bass_utils import run_bass_kernel_spmd

    emb_y = np.asarray(emb_y, dtype=np.float32)
    in_maps = _make_in_maps(
        {
            "emb_x": emb_x,
            "mu_w1": mu_w1,
            "mu_b1": mu_b1,
            "mu_w2": mu_w2,
            "lv_w1": lv_w1,
            "lv_b1": lv_b1,
            "lv_w2": lv_w2,
        }
    )

    nc = _get_nc()
    res = run_bass_kernel_spmd(nc, in_maps, list(range(NCORES)))

    b2mu = np.asarray(mu_b2, np.float64)  # (64,)
    b2lv = np.asarray(lv_b2, np.float64)
    B = np.zeros(DY)
    E = np.zeros(DY)
    A = 0.0
    C = 0.0
    for c in range(NCORES):
        yT = emb_y[c * NLOC : (c + 1) * NLOC].T.astype(np.float64)  # (64,1024)
        mu_st = res.results[c]["omu"].astype(np.float64)  # (128, 512)
        lv_st = res.results[c]["olv"].astype(np.float64)
        # unstack: partitions 0:64 = n cols 0:512, 64:128 = cols 512:1024
        mu = np.concatenate([mu_st[0:DY], mu_st[DY:]], axis=1) + b2mu[:, None]
        lv_raw = np.concatenate([lv_st[0:DY], lv_st[DY:]], axis=1)
        ivc = np.exp(-np.tanh(lv_raw + b2lv[:, None]))
        mic = mu * ivc
        B += ivc.sum(axis=1)
        E += mic.sum(axis=1)
        A += (ivc * yT**2).sum()
        C += (mic * yT).sum()

    y64 = emb_y.astype(np.float64)
    ybar = y64.mean(axis=0)
    y2bar = (y64**2).mean(axis=0)

    total = A - 2.0 * C + (2.0 * E * ybar - B * y2bar).sum()
    loss = -0.5 / N * total
    return np.float32(loss)
